# revision 5
# baseline (speedup 1.0000x reference)
"""Trainium2 Bass kernel for nn_IntraFreqAttention.

Per-(b,n) problem (32 total, 4 per core on 8 cores):
  x[S=1024, c=64] -> t = x @ W_in^T [S, 256]
  tn = LN1(t); score = tn @ tn^T (causal); p = softmax(score)
  att = p @ t + t; h = relu(LN2(att) @ W1^T) @ W2^T; y = h + t
  out = y @ W_out^T [S, 64]

On-chip layout strategy: feature-major ("d on partitions, tokens on free")
for all linear layers so weights are the stationary matmul operand and the
raw buffer slice [c, s] feeds linear_in with no transpose.  LayerNorms and
softmax run token-major (free-dim reductions); PE transposes (128x128 via
identity matmul) move tensors between the two layouts.  Softmax p is
transposed with a diag(1/denom) stationary operand, folding the softmax
normalization into the transpose for free.  All big matmuls run as
float32r (full fp32 data, 1 cycle/row at N>=256).
"""

import numpy as np
from contextlib import ExitStack

import concourse.bass as bass
import concourse.mybir as mybir
import concourse.tile as tile
from concourse import bacc
from concourse.bass import ts
from concourse.bass_utils import run_bass_kernel_spmd
from concourse.masks import make_identity, make_causal_mask

FP32 = mybir.dt.float32
FP32R = mybir.dt.float32r
AX = mybir.AxisListType
OP = mybir.AluOpType
AF = mybir.ActivationFunctionType

B, C, NFREQ, V, W = 4, 64, 8, 32, 32
S = V * W            # 1024 tokens per problem
D = 256              # model dim (SPA)
F = 512              # FF dim
NCORES = 8
NPROB = B * NFREQ    # 32 independent attention problems
PPC = NPROB // NCORES  # 4 problems per core
EPS = 1e-5
MASK_VAL = -1e15
NS = S // 128        # 8 s-tiles
ND = D // 128        # 2 d-tiles
NF = F // 128        # 4 f-tiles
NCH = S // 512       # 2 512-wide chunks


def r(ap):
    return ap.bitcast(FP32R)


def build_kernel():
    nc = bacc.Bacc("TRN2", target_bir_lowering=False, debug=False,
                   num_devices=NCORES)

    x_dram = nc.dram_tensor("x", [PPC, C, S], FP32, kind="ExternalInput").ap()
    wint_d = nc.dram_tensor("wint", [C, D], FP32, kind="ExternalInput").ap()
    w1t_d = nc.dram_tensor("w1t", [D, F], FP32, kind="ExternalInput").ap()
    w2t_d = nc.dram_tensor("w2t", [F, D], FP32, kind="ExternalInput").ap()
    wot_d = nc.dram_tensor("wot", [D, C], FP32, kind="ExternalInput").ap()
    ln1g_d = nc.dram_tensor("ln1g", [D], FP32, kind="ExternalInput").ap()
    ln1b_d = nc.dram_tensor("ln1b", [D], FP32, kind="ExternalInput").ap()
    ln2g_d = nc.dram_tensor("ln2g", [D], FP32, kind="ExternalInput").ap()
    ln2b_d = nc.dram_tensor("ln2b", [D], FP32, kind="ExternalInput").ap()
    y_dram = nc.dram_tensor("y", [PPC, C, S], FP32, kind="ExternalOutput").ap()

    with tile.TileContext(nc) as tc, ExitStack() as ctx:
        emit(ctx, tc, x_dram, wint_d, w1t_d, w2t_d, wot_d,
             ln1g_d, ln1b_d, ln2g_d, ln2b_d, y_dram)

    nc.compile()
    return nc


def emit(ctx, tc, x_dram, wint_d, w1t_d, w2t_d, wot_d,
         ln1g_d, ln1b_d, ln2g_d, ln2b_d, y_dram):
    nc = tc.nc

    consts = ctx.enter_context(tc.tile_pool(name="consts", bufs=1))
    xp = ctx.enter_context(tc.tile_pool(name="xp", bufs=2))
    big = ctx.enter_context(tc.tile_pool(name="big", bufs=2))
    pTp = ctx.enter_context(tc.tile_pool(name="pTp", bufs=1))
    relup = ctx.enter_context(tc.tile_pool(name="relup", bufs=1))
    ptilp = ctx.enter_context(tc.tile_pool(name="ptilp", bufs=2))
    scr = ctx.enter_context(tc.tile_pool(name="scr", bufs=4))
    small = ctx.enter_context(tc.tile_pool(name="small", bufs=32))
    outp = ctx.enter_context(tc.tile_pool(name="outp", bufs=2))

    ps_lin = ctx.enter_context(tc.tile_pool(name="ps_lin", bufs=2, space="PSUM"))
    ps_qk = ctx.enter_context(tc.tile_pool(name="ps_qk", bufs=2, space="PSUM"))
    ps_t = ctx.enter_context(tc.tile_pool(name="ps_t", bufs=2, space="PSUM"))
    ps_av = ctx.enter_context(tc.tile_pool(name="ps_av", bufs=2, space="PSUM"))

    # ---- constants: weights, LN params, identity, causal mask ----
    wint_sb = consts.tile([C, D], FP32R, tag="wint")          # [64, 256]
    nc.sync.dma_start(wint_sb[:], wint_d.bitcast(FP32R))
    w1t_sb = consts.tile([128, ND, F], FP32R, tag="w1t")      # [128, 2, 512]
    nc.sync.dma_start(w1t_sb[:], w1t_d.rearrange("(t p) f -> p t f", p=128).bitcast(FP32R))
    w2t_sb = consts.tile([128, NF, D], FP32R, tag="w2t")      # [128, 4, 256]
    nc.sync.dma_start(w2t_sb[:], w2t_d.rearrange("(t p) f -> p t f", p=128).bitcast(FP32R))
    wot_sb = consts.tile([128, ND, C], FP32R, tag="wot")      # [128, 2, 64]
    nc.sync.dma_start(wot_sb[:], wot_d.rearrange("(t p) f -> p t f", p=128).bitcast(FP32R))

    ln1g_sb = consts.tile([128, ND], FP32, tag="ln1g")
    nc.sync.dma_start(ln1g_sb[:], ln1g_d.rearrange("(t p) -> p t", p=128))
    ln1b_sb = consts.tile([128, ND], FP32, tag="ln1b")
    nc.sync.dma_start(ln1b_sb[:], ln1b_d.rearrange("(t p) -> p t", p=128))
    ln2g_sb = consts.tile([128, ND], FP32, tag="ln2g")
    nc.sync.dma_start(ln2g_sb[:], ln2g_d.rearrange("(t p) -> p t", p=128))
    ln2b_sb = consts.tile([128, ND], FP32, tag="ln2b")
    nc.sync.dma_start(ln2b_sb[:], ln2b_d.rearrange("(t p) -> p t", p=128))

    ident = consts.tile([128, 128], FP32, tag="ident")
    make_identity(nc, ident[:])
    causal = consts.tile([128, 128], FP32, tag="causal")
    make_causal_mask(nc, causal[:], mask_val=MASK_VAL)
    epst = consts.tile([128, 1], FP32, tag="epst")
    nc.vector.memset(epst[:], EPS)

    for p in range(PPC):
        # ---- load xT = buffer slice [c=64, s=1024] ----
        xT = xp.tile([C, S], FP32R, tag="xT")
        nc.sync.dma_start(xT[:], x_dram[p].bitcast(FP32R))

        # ---- t token-major [s, d]: 8 tiles; and LN1 -> tnT [d, s] ----
        t_sd = big.tile([128, NS, D], FP32, tag="t_sd")
        t_ds = big.tile([128, ND, S], FP32, tag="t_ds")
        tnT = big.tile([128, ND, S], FP32, tag="tnT")
        hT = big.tile([128, ND, S], FP32, tag="hT")
        y_ds = big.tile([128, ND, S], FP32, tag="y_ds")
        pT = pTp.tile([128, NS, S], FP32, tag="pT")
        relu1 = relup.tile([128, NF, S], FP32, tag="relu1")

        # t feature-major [d, s] straight from the raw layout
        for dt in range(ND):
            for ch in range(NCH):
                ps = ps_lin.tile([128, 512], FP32, tag="lin")
                nc.tensor.matmul(ps[:], r(wint_sb[:, ts(dt, 128)]),
                                 r(xT[:, ts(ch, 512)]), start=True, stop=True)
                nc.vector.tensor_copy(t_ds[:, dt, ts(ch, 512)], ps[:])

        for si in range(NS):
            ps = ps_lin.tile([128, D], FP32, tag="lin")
            nc.tensor.matmul(ps[:], r(xT[:, ts(si, 128)]), r(wint_sb[:]),
                             start=True, stop=True)
            nc.vector.tensor_copy(r(t_sd[:, si, :]), ps[:])

            # LN1 stats (token-major: free-dim reduction)
            st6 = small.tile([128, 6], FP32, tag="st6")
            nc.vector.bn_stats(st6[:], t_sd[:, si, :])
            mv = small.tile([128, 2], FP32, tag="mv")
            nc.vector.bn_aggr(mv[:], st6[:])
            rsig = small.tile([128, 1], FP32, tag="rsig")
            nc.scalar.activation(rsig[:], mv[:, 1:2], AF.Sqrt, bias=epst[:], scale=1.0)
            nc.vector.reciprocal(rsig[:], rsig[:])
            nmu = small.tile([128, 1], FP32, tag="nmu")
            nc.vector.scalar_tensor_tensor(nmu[:], mv[:, 0:1], -1.0, rsig[:],
                                           op0=OP.mult, op1=OP.mult)
            # tn = t * rsig - mu*rsig   (gamma/beta applied after transpose)
            tn = scr.tile([128, D], FP32, tag="tn")
            nc.scalar.activation(tn[:], t_sd[:, si, :], AF.Identity,
                                 bias=nmu[:], scale=rsig[:])
            for dt in range(ND):
                pst = ps_t.tile([128, 128], FP32, tag="tr")
                nc.tensor.transpose(pst[:], tn[:, ts(dt, 128)], ident[:])
                nc.vector.tensor_scalar(r(tnT[:, dt, ts(si, 128)]), pst[:],
                                        ln1g_sb[:, dt:dt + 1],
                                        ln1b_sb[:, dt:dt + 1],
                                        op0=OP.mult, op1=OP.add)

        # ---- attention: per q-block i ----
        for i in range(NS):
            wlen = (i + 1) * 128
            nch = (wlen + 511) // 512
            ptil = ptilp.tile([128, S], FP32, tag="ptil")
            esum = small.tile([128, 2], FP32, tag="esum")
            mx = small.tile([128, 2], FP32, tag="mx")
            qk_ps = []
            for ch in range(nch):
                lo = ch * 512
                cw = min(512, wlen - lo)
                ps = ps_qk.tile([128, 512], FP32, tag="qk")
                qk_ps.append((ps, lo, cw))
                for kt in range(ND):
                    nc.tensor.matmul(ps[:, :cw],
                                     r(tnT[:, kt, ts(i, 128)]),
                                     r(tnT[:, kt, lo:lo + cw]),
                                     start=(kt == 0), stop=(kt == ND - 1))
            # causal mask on the diagonal block (last 128 cols of window)
            ps, lo, cw = qk_ps[-1]
            nc.vector.tensor_add(ps[:, cw - 128:cw], ps[:, cw - 128:cw], causal[:])
            # row max -> negated shift
            for ch, (ps, lo, cw) in enumerate(qk_ps):
                nc.vector.tensor_reduce(mx[:, ch:ch + 1], ps[:, :cw],
                                        axis=AX.X, op=OP.max)
            negm = small.tile([128, 1], FP32, tag="negm")
            if nch > 1:
                nc.vector.tensor_reduce(negm[:], mx[:, :nch], axis=AX.X,
                                        op=OP.max)
                nc.vector.tensor_scalar_mul(negm[:], negm[:], -1.0)
            else:
                nc.vector.tensor_scalar_mul(negm[:], mx[:, 0:1], -1.0)
            # exp (+ per-chunk row sums via accum_out)
            for ch, (ps, lo, cw) in enumerate(qk_ps):
                nc.scalar.activation(ptil[:, lo:lo + cw], ps[:, :cw], AF.Exp,
                                     bias=negm[:], scale=1.0,
                                     accum_out=esum[:, ch:ch + 1])
            rden = small.tile([128, 1], FP32, tag="rden")
            if nch > 1:
                nc.vector.tensor_add(rden[:], esum[:, 0:1], esum[:, 1:2])
                nc.vector.reciprocal(rden[:], rden[:])
            else:
                nc.vector.reciprocal(rden[:], esum[:, 0:1])
            # diag(1/denom): fold softmax normalization into the transpose
            diag = scr.tile([128, 128], FP32, tag="diag")
            nc.vector.tensor_scalar_mul(diag[:], ident[:], rden[:])
            for j in range(i + 1):
                pst = ps_t.tile([128, 128], FP32, tag="tr")
                nc.tensor.transpose(pst[:], ptil[:, ts(j, 128)], diag[:])
                nc.vector.tensor_copy(r(pT[:, j, ts(i, 128)]), pst[:])
            # PV: att^T-free accumulation, token-major out [q, d]
            av = ps_av.tile([128, D], FP32, tag="av")
            for j in range(i + 1):
                nc.tensor.matmul(av[:], r(pT[:, j, ts(i, 128)]),
                                 r(t_sd[:, j, :]),
                                 start=(j == 0), stop=(j == i))
            att = scr.tile([128, D], FP32, tag="att")
            nc.vector.tensor_add(att[:], av[:], t_sd[:, i, :])

            # LN2 (token-major) -> transpose -> hT [d, s] with gamma/beta
            st6 = small.tile([128, 6], FP32, tag="st6")
            nc.vector.bn_stats(st6[:], att[:])
            mv = small.tile([128, 2], FP32, tag="mv")
            nc.vector.bn_aggr(mv[:], st6[:])
            rsig = small.tile([128, 1], FP32, tag="rsig")
            nc.scalar.activation(rsig[:], mv[:, 1:2], AF.Sqrt, bias=epst[:], scale=1.0)
            nc.vector.reciprocal(rsig[:], rsig[:])
            nmu = small.tile([128, 1], FP32, tag="nmu")
            nc.vector.scalar_tensor_tensor(nmu[:], mv[:, 0:1], -1.0, rsig[:],
                                           op0=OP.mult, op1=OP.mult)
            tn2 = scr.tile([128, D], FP32, tag="tn2")
            nc.scalar.activation(tn2[:], att[:], AF.Identity,
                                 bias=nmu[:], scale=rsig[:])
            for dt in range(ND):
                pst = ps_t.tile([128, 128], FP32, tag="tr")
                nc.tensor.transpose(pst[:], tn2[:, ts(dt, 128)], ident[:])
                nc.vector.tensor_scalar(r(hT[:, dt, ts(i, 128)]), pst[:],
                                        ln2g_sb[:, dt:dt + 1],
                                        ln2b_sb[:, dt:dt + 1],
                                        op0=OP.mult, op1=OP.add)

        # ---- FF1: relu(hT^T @ W1^T) feature-major [f, s] ----
        for ft in range(NF):
            for ch in range(NCH):
                ps = ps_lin.tile([128, 512], FP32, tag="lin")
                for dt in range(ND):
                    nc.tensor.matmul(ps[:], r(w1t_sb[:, dt, ts(ft, 128)]),
                                     r(hT[:, dt, ts(ch, 512)]),
                                     start=(dt == 0), stop=(dt == ND - 1))
                nc.vector.tensor_scalar_max(r(relu1[:, ft, ts(ch, 512)]), ps[:], 0.0)

        # ---- FF2 + residual with t: y = relu1^T @ W2^T + t, [d, s] ----
        for dt in range(ND):
            for ch in range(NCH):
                ps = ps_lin.tile([128, 512], FP32, tag="lin")
                for ft in range(NF):
                    nc.tensor.matmul(ps[:], r(w2t_sb[:, ft, ts(dt, 128)]),
                                     r(relu1[:, ft, ts(ch, 512)]),
                                     start=(ft == 0), stop=(ft == NF - 1))
                nc.vector.tensor_add(r(y_ds[:, dt, ts(ch, 512)]), ps[:],
                                     t_ds[:, dt, ts(ch, 512)])

        # ---- linear_out: [c=64, s] — matches raw output layout ----
        out_sb = outp.tile([C, S], FP32, tag="out")
        for ch in range(NCH):
            ps = ps_lin.tile([128, 512], FP32, tag="lin")
            for dt in range(ND):
                nc.tensor.matmul(ps[:C, :], r(wot_sb[:, dt, :]),
                                 r(y_ds[:, dt, ts(ch, 512)]),
                                 start=(dt == 0), stop=(dt == ND - 1))
            nc.scalar.copy(out_sb[:, ts(ch, 512)], ps[:C, :])
        nc.sync.dma_start(y_dram[p], out_sb[:])


_NC_CACHE = {}


def _get_nc():
    if "nc" not in _NC_CACHE:
        _NC_CACHE["nc"] = build_kernel()
    return _NC_CACHE["nc"]


def _make_in_maps(buffer, W_in, ln1_g, ln1_b, ln2_g, ln2_b, W1, W2, W_out):
    # buffer [b, c, n, v, w] -> per-problem xT [c, s]; problems = (b, n)
    xT = np.ascontiguousarray(
        np.transpose(np.asarray(buffer, np.float32), (0, 2, 1, 3, 4))
    ).reshape(NPROB, C, S)
    com = {
        "wint": np.ascontiguousarray(np.asarray(W_in, np.float32).T),
        "w1t": np.ascontiguousarray(np.asarray(W1, np.float32).T),
        "w2t": np.ascontiguousarray(np.asarray(W2, np.float32).T),
        "wot": np.ascontiguousarray(np.asarray(W_out, np.float32).T),
        "ln1g": np.asarray(ln1_g, np.float32),
        "ln1b": np.asarray(ln1_b, np.float32),
        "ln2g": np.asarray(ln2_g, np.float32),
        "ln2b": np.asarray(ln2_b, np.float32),
    }
    return [
        dict(com, x=np.ascontiguousarray(xT[c * PPC:(c + 1) * PPC]))
        for c in range(NCORES)
    ]


def run_on_hw(in_maps, **kwargs):
    nc = _get_nc()
    return run_bass_kernel_spmd(nc, in_maps, core_ids=list(range(NCORES)),
                                **kwargs)


def kernel(buffer, W_in, ln1_g, ln1_b, ln2_g, ln2_b, W1, W2, W_out):
    in_maps = _make_in_maps(buffer, W_in, ln1_g, ln1_b, ln2_g, ln2_b,
                            W1, W2, W_out)
    res = run_on_hw(in_maps)
    out = np.concatenate([res.results[c]["y"] for c in range(NCORES)], axis=0)
    # [32, c, s] -> [b, n, c, v, w] -> [b, c, n, v, w]
    out = out.reshape(B, NFREQ, C, V, W).transpose(0, 2, 1, 3, 4)
    return np.ascontiguousarray(out.astype(np.float32))
